# revision 4
# baseline (speedup 1.0000x reference)
"""Grouped conv2d (DynamicConv2D) Bass kernel for 8 Trainium2 NeuronCores.

Problem: x [1, B*C, H, W] (B=32 samples folded into channels, C=64),
kernels [B*C, C, 3, 3], grouped conv with groups=B, SAME padding.

Sharding: data-parallel over samples — core c handles samples 4c..4c+3
(channels 256c..256c+255 of x and of the output). No cross-core comms.

Per-core kernel strategy:
  * Two samples ("A", "B") are processed together: A's 64 input channels
    live in SBUF partitions 0-63, B's in 64-127.
  * Output tile = 2 output rows x 256 cols = 512 positions (one PSUM bank).
    For each of the 9 filter taps, a K=64/M=64/N=512 matmul accumulates
    into PSUM.  Two adjacent tiles (t, t+1) are interleaved so the 4
    matmuls of a tap occupy the 4 disjoint 64x64 quadrants of the PE
    array (tile_position derived from operand base partitions) and run
    concurrently:
        (0,0):   A(t)   -> ps_t[0:64]
        (0,64):  A(t+1) -> ps_{t+1}[64:128]
        (64,0):  B(t+1) -> ps_{t+1}[0:64]
        (64,64): B(t)   -> ps_t[64:128]
    Each tile's A/B results land in the two partition halves of ONE psum
    bank, so evacuation is a single 128-lane copy to SBUF.
  * x is zero-padded (H+2, W+2) and converted on the host, so tap shifts
    are plain AP offsets and no device-side memsets are needed.
  * Weights are pre-transposed on the host to lhsT[c_in, sp, tap, c_out].
"""

import numpy as np
import ml_dtypes

import concourse.bass as bass
import concourse.tile as tile
from concourse import bacc, mybir
from concourse.bass_utils import run_bass_kernel_spmd

N_CORES = 8
B = 32
C = 64          # per-sample in/out channels
H = W = 256
HP, WP = H + 2, W + 2
S_PER_CORE = B // N_CORES          # 4 samples per core
SP_PER_CORE = S_PER_CORE // 2      # 2 sample-pairs per core
CH_PER_CORE = S_PER_CORE * C       # 256 channels per core

CHUNK_ROWS = 32                    # output rows per chunk
N_CHUNKS = H // CHUNK_ROWS         # 8
TP_PER_CHUNK = CHUNK_ROWS // 4     # 8 tile-pairs (tile = 2 rows)

USE_BF16 = True
DT_IN = mybir.dt.bfloat16 if USE_BF16 else mybir.dt.float32
NP_IN = ml_dtypes.bfloat16 if USE_BF16 else np.float32


def build_program(reps: int = 1):
    """Build the per-core Bass program (same program for all 8 cores)."""
    nc = bacc.Bacc(
        "TRN2", target_bir_lowering=False, debug=False, num_devices=N_CORES
    )
    x_d = nc.dram_tensor("x", [CH_PER_CORE, HP, WP], DT_IN, kind="ExternalInput")
    w_d = nc.dram_tensor(
        "w", [128, SP_PER_CORE, 9, C], DT_IN, kind="ExternalInput"
    )
    o_d = nc.dram_tensor(
        "out", [CH_PER_CORE, H, W], mybir.dt.float32, kind="ExternalOutput"
    )

    with tile.TileContext(nc) as tc:
        with (
            tc.tile_pool(name="wpool", bufs=1) as wpool,
            tc.tile_pool(name="xpool", bufs=3) as xpool,
            tc.tile_pool(name="opool", bufs=2) as opool,
            tc.tile_pool(name="pspool", bufs=8, space=bass.MemorySpace.PSUM) as pspool,
        ):
            w_sb = wpool.tile([128, SP_PER_CORE, 9, C], DT_IN)
            nc.sync.dma_start(w_sb[:], w_d[:])

            for _rep in range(reps):
                for sp in range(SP_PER_CORE):
                    for ch in range(N_CHUNKS):
                        r0 = ch * CHUNK_ROWS
                        x_sb = xpool.tile([128, CHUNK_ROWS + 2, WP], DT_IN)
                        nc.sync.dma_start(
                            x_sb[:],
                            x_d[sp * 128 : (sp + 1) * 128, r0 : r0 + CHUNK_ROWS + 2, :],
                        )
                        o_sb = opool.tile(
                            [128, CHUNK_ROWS, W], mybir.dt.float32
                        )
                        for tp in range(TP_PER_CHUNK):
                            h0 = 4 * tp  # first output row (in chunk) of tile t
                            ps_a = pspool.tile([128, 2, W], mybir.dt.float32, tag="ps")
                            ps_b = pspool.tile([128, 2, W], mybir.dt.float32, tag="ps")
                            for k in range(9):
                                kh, kw = divmod(k, 3)
                                st = k == 0
                                sto = k == 8
                                wa = w_sb[0:64, sp, k, :]
                                wb = w_sb[64:128, sp, k, :]
                                ra = x_sb[0:64, h0 + kh : h0 + kh + 2, kw : kw + W]
                                ra2 = x_sb[0:64, h0 + 2 + kh : h0 + 4 + kh, kw : kw + W]
                                rb = x_sb[64:128, h0 + kh : h0 + kh + 2, kw : kw + W]
                                rb2 = x_sb[
                                    64:128, h0 + 2 + kh : h0 + 4 + kh, kw : kw + W
                                ]
                                # 4 disjoint PE quadrants -> concurrent
                                nc.tensor.matmul(
                                    ps_a[0:64], wa, ra, start=st, stop=sto, skip_group_check=True
                                )
                                nc.tensor.matmul(
                                    ps_b[64:128], wa, ra2, start=st, stop=sto, skip_group_check=True
                                )
                                nc.tensor.matmul(
                                    ps_b[0:64], wb, rb2, start=st, stop=sto, skip_group_check=True
                                )
                                nc.tensor.matmul(
                                    ps_a[64:128], wb, rb, start=st, stop=sto, skip_group_check=True
                                )
                            nc.any.tensor_copy(
                                o_sb[:, h0 : h0 + 2, :], ps_a[:]
                            )
                            # ps_b halves are swapped (A at 64:128):
                            # evacuate with two cross-half copies
                            nc.vector.tensor_copy(
                                o_sb[0:64, h0 + 2 : h0 + 4, :], ps_b[64:128]
                            )
                            nc.vector.tensor_copy(
                                o_sb[64:128, h0 + 2 : h0 + 4, :], ps_b[0:64]
                            )
                        nc.sync.dma_start(
                            o_d[
                                sp * 128 : (sp + 1) * 128,
                                r0 : r0 + CHUNK_ROWS,
                                :,
                            ],
                            o_sb[:],
                        )
    nc.compile()
    return nc


def prep_x(x: np.ndarray) -> np.ndarray:
    """[1, B*C, H, W] f32 -> padded [B*C, HP, WP] in input dtype."""
    x = np.ascontiguousarray(x.reshape(B * C, H, W))
    xp = np.zeros((B * C, HP, WP), dtype=NP_IN)
    xp[:, 1 : H + 1, 1 : W + 1] = x
    return xp


def prep_w(kernels: np.ndarray) -> np.ndarray:
    """[B*C, C, 3, 3] f32 -> per-core lhsT [8, 128, SP, 9, C]."""
    k = kernels.reshape(B, C, C, 3, 3)          # [s, c_out, c_in, kh, kw]
    wt = np.transpose(k, (2, 0, 3, 4, 1))        # [c_in, s, kh, kw, c_out]
    wt = np.ascontiguousarray(wt).reshape(C, B, 9, C).astype(NP_IN)
    w_all = np.zeros((N_CORES, 128, SP_PER_CORE, 9, C), dtype=NP_IN)
    for c in range(N_CORES):
        for sp in range(SP_PER_CORE):
            s_a = S_PER_CORE * c + 2 * sp
            w_all[c, 0:64, sp] = wt[:, s_a]
            w_all[c, 64:128, sp] = wt[:, s_a + 1]
    return w_all


def make_in_maps(x: np.ndarray, kernels: np.ndarray):
    xp = prep_x(x)
    w_all = prep_w(kernels)
    in_maps = []
    for c in range(N_CORES):
        in_maps.append(
            {
                "x": np.ascontiguousarray(
                    xp[c * CH_PER_CORE : (c + 1) * CH_PER_CORE]
                ),
                "w": w_all[c],
            }
        )
    return in_maps


_NC_CACHE = {}


def kernel(x: np.ndarray, kernels: np.ndarray, batch_size=None) -> np.ndarray:
    assert x.shape == (1, B * C, H, W), x.shape
    assert kernels.shape == (B * C, C, 3, 3), kernels.shape
    if "nc" not in _NC_CACHE:
        _NC_CACHE["nc"] = build_program()
    nc = _NC_CACHE["nc"]
    in_maps = make_in_maps(np.asarray(x), np.asarray(kernels))
    res = run_bass_kernel_spmd(nc, in_maps, core_ids=list(range(N_CORES)))
    out = np.empty((1, B * C, H, W), dtype=np.float32)
    for c in range(N_CORES):
        out[0, c * CH_PER_CORE : (c + 1) * CH_PER_CORE] = res.results[c]["out"]
    return out


# revision 8
# speedup vs baseline: 1.3845x; 1.3845x over previous
"""Grouped conv2d (DynamicConv2D) Bass kernel for 8 Trainium2 NeuronCores.

Problem: x [1, B*C, H, W] (B=32 samples folded into channels, C=64),
kernels [B*C, C, 3, 3], grouped conv with groups=B, SAME padding.

Sharding: data-parallel over samples — core c handles samples 4c..4c+3
(channels 256c..256c+255 of x and of the output). No cross-core comms.

Per-core kernel strategy:
  * Two samples ("A", "B") are processed together: A's 64 input channels
    live in SBUF partitions 0-63, B's in 64-127.
  * Output tile = 2 output rows x 256 cols = 512 positions (one PSUM bank).
    For each of the 9 filter taps, a K=64/M=64/N=512 matmul accumulates
    into PSUM.  Two adjacent tiles (t, t+1) are interleaved so the 4
    matmuls of a tap occupy the 4 disjoint 64x64 quadrants of the PE
    array (tile_position derived from operand base partitions) and run
    concurrently:
        (0,0):   A(t)   -> ps_t[0:64]
        (0,64):  A(t+1) -> ps_{t+1}[64:128]
        (64,0):  B(t+1) -> ps_{t+1}[0:64]
        (64,64): B(t)   -> ps_t[64:128]
    Each tile's A/B results land in the two partition halves of ONE psum
    bank, so evacuation is a single 128-lane copy to SBUF.
  * x is zero-padded (H+2, W+2) and converted on the host, so tap shifts
    are plain AP offsets and no device-side memsets are needed.
  * Weights are pre-transposed on the host to lhsT[c_in, sp, tap, c_out].
"""

import numpy as np
import ml_dtypes

import concourse.bass as bass
import concourse.tile as tile
from concourse import bacc, mybir
from concourse.bass_utils import run_bass_kernel_spmd

N_CORES = 8
B = 32
C = 64          # per-sample in/out channels
H = W = 256
HP, WP = H + 2, W + 2
S_PER_CORE = B // N_CORES          # 4 samples per core
SP_PER_CORE = S_PER_CORE // 2      # 2 sample-pairs per core
CH_PER_CORE = S_PER_CORE * C       # 256 channels per core

CHUNK_ROWS = 32                    # output rows per chunk
N_CHUNKS = H // CHUNK_ROWS         # 8
TP_PER_CHUNK = CHUNK_ROWS // 4     # 8 tile-pairs (tile = 2 rows)

USE_BF16 = True
DT_IN = mybir.dt.bfloat16 if USE_BF16 else mybir.dt.float32
NP_IN = ml_dtypes.bfloat16 if USE_BF16 else np.float32
OUT_BF16 = True
DT_OUT = mybir.dt.bfloat16 if OUT_BF16 else mybir.dt.float32
NP_OUT = ml_dtypes.bfloat16 if OUT_BF16 else np.float32


def build_program(reps: int = 1):
    """Build the per-core Bass program (same program for all 8 cores).

    reps > 1 wraps the whole computation in a hardware For_i loop (used
    only by test.py for precise timing; the graded path uses reps=1,
    which emits no loop instructions at all).
    """
    nc = bacc.Bacc(
        "TRN2", target_bir_lowering=False, debug=False, num_devices=N_CORES
    )
    x_d = nc.dram_tensor("x", [CH_PER_CORE, HP, WP], DT_IN, kind="ExternalInput")
    w_d = nc.dram_tensor(
        "w", [128, SP_PER_CORE, 9, C], DT_IN, kind="ExternalInput"
    )
    o_d = nc.dram_tensor(
        "out", [CH_PER_CORE, H, W], DT_OUT, kind="ExternalOutput"
    )

    with tile.TileContext(nc) as tc:
        with (
            tc.tile_pool(name="wpool", bufs=1) as wpool,
            tc.tile_pool(name="xpool", bufs=3) as xpool,
            tc.tile_pool(name="opool", bufs=2) as opool,
            tc.tile_pool(name="pspool", bufs=8, space=bass.MemorySpace.PSUM) as pspool,
        ):
            w_sb = wpool.tile([128, SP_PER_CORE, 9, C], DT_IN)
            nc.sync.dma_start(w_sb[:], w_d[:])

            def body():
                for sp in range(SP_PER_CORE):
                    for ch in range(N_CHUNKS):
                        r0 = ch * CHUNK_ROWS
                        x_sb = xpool.tile([128, CHUNK_ROWS + 2, WP], DT_IN)
                        nc.sync.dma_start(
                            x_sb[:],
                            x_d[sp * 128 : (sp + 1) * 128, r0 : r0 + CHUNK_ROWS + 2, :],
                        )
                        o_sb = opool.tile([128, CHUNK_ROWS, W], DT_OUT)
                        for tp in range(TP_PER_CHUNK):
                            h0 = 4 * tp  # first output row (in chunk) of tile t
                            ps_a = pspool.tile([128, 2, W], mybir.dt.float32, tag="ps")
                            ps_b = pspool.tile([128, 2, W], mybir.dt.float32, tag="ps")
                            for k in range(9):
                                kh, kw = divmod(k, 3)
                                st = k == 0
                                sto = k == 8
                                wa = w_sb[0:64, sp, k, :]
                                wb = w_sb[64:128, sp, k, :]
                                ra = x_sb[0:64, h0 + kh : h0 + kh + 2, kw : kw + W]
                                ra2 = x_sb[0:64, h0 + 2 + kh : h0 + 4 + kh, kw : kw + W]
                                rb = x_sb[64:128, h0 + kh : h0 + kh + 2, kw : kw + W]
                                rb2 = x_sb[
                                    64:128, h0 + 2 + kh : h0 + 4 + kh, kw : kw + W
                                ]
                                # 4 disjoint PE quadrants -> concurrent
                                nc.tensor.matmul(
                                    ps_a[0:64], wa, ra, start=st, stop=sto, skip_group_check=True
                                )
                                nc.tensor.matmul(
                                    ps_b[64:128], wa, ra2, start=st, stop=sto, skip_group_check=True
                                )
                                nc.tensor.matmul(
                                    ps_b[0:64], wb, rb2, start=st, stop=sto, skip_group_check=True
                                )
                                nc.tensor.matmul(
                                    ps_a[64:128], wb, rb, start=st, stop=sto, skip_group_check=True
                                )
                            nc.any.tensor_copy(
                                o_sb[:, h0 : h0 + 2, :], ps_a[:]
                            )
                            # ps_b halves are swapped (A at 64:128):
                            # evacuate with two cross-half copies
                            nc.vector.tensor_copy(
                                o_sb[0:64, h0 + 2 : h0 + 4, :], ps_b[64:128]
                            )
                            nc.vector.tensor_copy(
                                o_sb[64:128, h0 + 2 : h0 + 4, :], ps_b[0:64]
                            )
                        nc.sync.dma_start(
                            o_d[
                                sp * 128 : (sp + 1) * 128,
                                r0 : r0 + CHUNK_ROWS,
                                :,
                            ],
                            o_sb[:],
                        )

            if reps == 1:
                body()
            else:
                with tc.For_i(0, reps, 1):
                    body()
    nc.compile()
    return nc


def prep_x(x: np.ndarray) -> np.ndarray:
    """[1, B*C, H, W] f32 -> padded [B*C, HP, WP] in input dtype."""
    x = np.ascontiguousarray(x.reshape(B * C, H, W))
    xp = np.zeros((B * C, HP, WP), dtype=NP_IN)
    xp[:, 1 : H + 1, 1 : W + 1] = x
    return xp


def prep_w(kernels: np.ndarray) -> np.ndarray:
    """[B*C, C, 3, 3] f32 -> per-core lhsT [8, 128, SP, 9, C]."""
    k = kernels.reshape(B, C, C, 3, 3)          # [s, c_out, c_in, kh, kw]
    wt = np.transpose(k, (2, 0, 3, 4, 1))        # [c_in, s, kh, kw, c_out]
    wt = np.ascontiguousarray(wt).reshape(C, B, 9, C).astype(NP_IN)
    w_all = np.zeros((N_CORES, 128, SP_PER_CORE, 9, C), dtype=NP_IN)
    for c in range(N_CORES):
        for sp in range(SP_PER_CORE):
            s_a = S_PER_CORE * c + 2 * sp
            w_all[c, 0:64, sp] = wt[:, s_a]
            w_all[c, 64:128, sp] = wt[:, s_a + 1]
    return w_all


def make_in_maps(x: np.ndarray, kernels: np.ndarray):
    xp = prep_x(x)
    w_all = prep_w(kernels)
    in_maps = []
    for c in range(N_CORES):
        in_maps.append(
            {
                "x": np.ascontiguousarray(
                    xp[c * CH_PER_CORE : (c + 1) * CH_PER_CORE]
                ),
                "w": w_all[c],
            }
        )
    return in_maps


_NC_CACHE = {}


def kernel(x: np.ndarray, kernels: np.ndarray, batch_size=None) -> np.ndarray:
    assert x.shape == (1, B * C, H, W), x.shape
    assert kernels.shape == (B * C, C, 3, 3), kernels.shape
    if "nc" not in _NC_CACHE:
        _NC_CACHE["nc"] = build_program()
    nc = _NC_CACHE["nc"]
    in_maps = make_in_maps(np.asarray(x), np.asarray(kernels))
    res = run_bass_kernel_spmd(nc, in_maps, core_ids=list(range(N_CORES)))
    out = np.empty((1, B * C, H, W), dtype=np.float32)
    for c in range(N_CORES):
        out[0, c * CH_PER_CORE : (c + 1) * CH_PER_CORE] = res.results[c][
            "out"
        ].astype(np.float32)
    return out
